# revision 49
# baseline (speedup 1.0000x reference)
"""Trainium2 Bass kernel for sliding-window GQA attention block.

Reference computation (B=2, S=4096, DIM=1024, H=16 q-heads, KV=2 kv-heads,
D=64, W=256 window):
    q = x@Wq + bq ; k = x@Wk + bk ; v = x@Wv + bv        (GQA repeat kv x8)
    local attention: query t attends keys [t-128, t+128) (zero-padded edges,
    no 1/sqrt(d) scaling), softmax, out = probs@v
    y = out@Wo + bo   (bo is added on the host after the gather)

Sharding: 8 cores = batch(2) x seq-quarter(4). Each core computes 1024
query rows end-to-end (all 16 heads) from a 1280-row haloed x slice.
No cross-core communication; host pads/transposes/gathers.

On-device pipeline per core (all matmuls bf16, fp32 PSUM accumulation):
  QKV projections (Q bias via per-partition add during the PSUM->SBUF
  copy; K/V biases via K=1 indicator-row matmuls that also zero K,V at
  padded halo rows) -> scores computed directly TRANSPOSED (S^T[u,t],
  keys on partitions; the two kv-halves of each key-chunk row-packed via
  tile_position so they run concurrently on the PE, each into its own
  1-bank PSUM tile) -> per-half exp on ScalarE (its only job; halves the
  exp->PV latency quantum) -> 0/1 band-mask multiply on VectorE for the
  two triangular chunks only -> probs @ [V|1] grouped 4 heads per PSUM
  bank (the ones column appended to V emits the softmax denominator at
  column 64 of each head's 65-col group) -> one strided reciprocal + one
  partition-broadcast tensor_tensor multiply normalizes 4 heads at once
  -> PE-transpose of the 128x128 attn blocks -> out-projection streamed
  per 512-col half with its own output DMA.

Scheduling: input DMAs are big multi-chunk transfers spread over the 3
DMA-capable queues, ordered so K/V-projection data lands first; Wq is
stored m-major on the host and arrives as 8 contiguous per-tile slices
so each qT projection starts as soon as its own slice lands. Identity
comes from the host; band masks build on gpsimd AFTER its DMA issues.
Self-contained junk matmuls warm the PE HAM clock gate during the DMA
wait. The pre-attention projections borrow the (then idle) attention
score PSUM banks for a 4-deep pipeline. Inside the attention loop the
out-projection of tile mt-1 and the remaining JIT projections run as
fillers between the score matmuls and PV of each head-group, covering
the serial exp latency; only out_proj(7) remains after the last tile.
"""

import functools
import numpy as np

B, S, DIM = 2, 4096, 1024
H, KV, D = 16, 2, 64
W, HW = 256, 128
NCORES = 8
QT = 4           # sequence quarters
T = S // QT      # 1024 query rows per core
TH = T + 2 * HW  # 1280 haloed rows
XSPLIT = 640     # xT column slab boundary


@functools.lru_cache(maxsize=1)
def _build_nc():
    import concourse.bacc as bacc
    import concourse.tile as tile
    from concourse import mybir

    f32 = mybir.dt.float32
    bf16 = mybir.dt.bfloat16
    Exp = mybir.ActivationFunctionType.Exp

    nc = bacc.Bacc("TRN2", target_bir_lowering=False, debug=False)

    xT = nc.dram_tensor("xT", [DIM, TH], bf16, kind="ExternalInput")
    wq = nc.dram_tensor("Wq", [DIM, DIM], bf16, kind="ExternalInput")
    wkv = nc.dram_tensor("Wkv", [DIM, 2 * KV * D], bf16, kind="ExternalInput")
    wo = nc.dram_tensor("Wo", [DIM, DIM], bf16, kind="ExternalInput")
    bqc = nc.dram_tensor("bqc", [128, 8], f32, kind="ExternalInput")
    # [bk (128) | bv (128) | bo (1024) | ind (1280)]
    crow = nc.dram_tensor("crow", [1, 2560], bf16, kind="ExternalInput")
    identd = nc.dram_tensor("identd", [128, 128], bf16, kind="ExternalInput")
    out = nc.dram_tensor("out", [T, DIM], bf16, kind="ExternalOutput")

    with tile.TileContext(nc) as tc:
        with tc.tile_pool(name="const", bufs=1) as const, \
             tc.tile_pool(name="w", bufs=1) as wpool, \
             tc.tile_pool(name="act", bufs=1) as actp, \
             tc.tile_pool(name="attn", bufs=2) as attnp, \
             tc.tile_pool(name="ps", bufs=2, space="PSUM") as ps:

            # ---- PE warmup: keeps the HAM clock gate open while the input
            # DMAs stream. Uses a locally-memset zero tile so the warmup has
            # no DMA dependency and starts right after the preamble. --------
            junk_rhs = const.tile([128, 512], bf16, tag="junk_rhs")
            nc.vector.memset(junk_rhs, 0.0)
            junk_ps = ps.tile([128, 512], f32, tag="proj", name="junk")
            for _ in range(28):
                nc.tensor.matmul(out=junk_ps, lhsT=junk_rhs[:, 0:128],
                                 rhs=junk_rhs, start=True, stop=True)

            # ---- DMAs: few big multi-chunk transfers, explicit queues -----
            bq_sb = const.tile([128, 8], f32, tag="bq")
            crow_sb = const.tile([1, 2560], bf16, tag="crow")
            bkr = crow_sb[:, 0:KV * D]
            bvr = crow_sb[:, KV * D:2 * KV * D]
            ind_sb = crow_sb[:, 1280:1280 + TH]

            wkv_big = wpool.tile([128, 8 * 2 * KV * D], bf16, tag="wkv")
            wkv_v = wkv_big.rearrange("p (k c) -> p k c", k=8)
            wk_sb = [wkv_v[:, k, 0:KV * D] for k in range(8)]
            wv_sb = [wkv_v[:, k, KV * D:2 * KV * D] for k in range(8)]
            xT_big = wpool.tile([128, 8 * TH], bf16, tag="xT")
            xT_v = xT_big.rearrange("p (k c) -> p k c", k=8)
            xT_sb = [xT_v[:, k, :] for k in range(8)]
            # per-m Wq tiles: both DMA sides fully contiguous (cheap issue)
            wq_sbm = [wpool.tile([128, 8 * 128], bf16, tag=f"wq{m}",
                                 name=f"wq{m}") for m in range(8)]
            wo_big = wpool.tile([128, 8 * DIM], bf16, tag="wo")
            wo_v = wo_big.rearrange("p (k c) -> p k c", k=8)
            wo_sb = [wo_v[:, k, :] for k in range(8)]

            xT_r = xT.rearrange("(k p) c -> p k c", k=8)
            wo_r = wo.rearrange("(k p) c -> p k c", k=8)
            wkv_r = wkv.rearrange("(k p) c -> p k c", k=8)

            # Wq arrives as per-m column slices so each qT tile's projection
            # can start as soon as its own 256KB lands (overlaps the DMA).
            # The host stores Wq m-major ([m, p, k, c]) so each slice is one
            # fully contiguous 256KB read with 2KB per-partition lines.
            wq_mr = wq.rearrange("(m p) c -> m p c", m=8)

            def wq_m(eng, m):
                eng.dma_start(out=wq_sbm[m], in_=wq_mr[m])

            # sync queue: xT slab halves + wq m-slices + slab1 + wo (k 0-3)
            nc.sync.dma_start(out=xT_v[:, 0:4, 0:XSPLIT],
                              in_=xT_r[:, 0:4, 0:XSPLIT])
            wq_m(nc.sync, 0)
            wq_m(nc.sync, 2)
            wq_m(nc.sync, 4)
            nc.sync.dma_start(out=xT_v[:, 0:4, XSPLIT:TH],
                              in_=xT_r[:, 0:4, XSPLIT:TH])
            nc.sync.dma_start(out=wo_v[:, 0:4, :], in_=wo_r[:, 0:4, :])
            # scalar queue: the other halves
            nc.scalar.dma_start(out=xT_v[:, 4:8, 0:XSPLIT],
                                in_=xT_r[:, 4:8, 0:XSPLIT])
            wq_m(nc.scalar, 1)
            wq_m(nc.scalar, 3)
            wq_m(nc.scalar, 5)
            nc.scalar.dma_start(out=xT_v[:, 4:8, XSPLIT:TH],
                                in_=xT_r[:, 4:8, XSPLIT:TH])
            nc.scalar.dma_start(out=wo_v[:, 4:8, :], in_=wo_r[:, 4:8, :])
            # gpsimd queue: small constants + wkv, then the mask build below
            ident = const.tile([128, 128], bf16, tag="ident")
            nc.gpsimd.dma_start(out=ident, in_=identd[:, :])
            nc.gpsimd.dma_start(out=crow_sb, in_=crow[:, :])
            nc.gpsimd.dma_start(out=bq_sb, in_=bqc[:, :])
            nc.gpsimd.dma_start(out=wkv_v[:, :, :], in_=wkv_r)
            # gpsimd is done by ~13us while sync/scalar still stream: give
            # it the last two Wq slices so the attention start isn't paced
            # by a 5th transfer on the busy queues
            wq_m(nc.gpsimd, 6)
            wq_m(nc.gpsimd, 7)

            # 0/1 window masks, transposed (key r, query c) orientation, for
            # the two triangular chunks; both kv-halves side by side.
            # j=0 chunk: valid where r >= c; j=2 chunk: valid where r < c.
            # Built on gpsimd AFTER its DMA issues (the engine is idle then).
            masks = const.tile([128, 1024], bf16, tag="masks")
            mask_lo = masks[:, 0:512]
            mask_hi = masks[:, 512:1024]
            nc.gpsimd.memset(mask_lo, 1.0)
            nc.gpsimd.memset(mask_hi, 1.0)
            for blk2 in range(0, 512, 128):
                nc.gpsimd.affine_select(
                    out=mask_lo[:, blk2:blk2 + 128],
                    in_=mask_lo[:, blk2:blk2 + 128],
                    compare_op=mybir.AluOpType.is_ge,
                    fill=0.0, base=0, pattern=[[-1, 128]],
                    channel_multiplier=1)
                nc.gpsimd.affine_select(
                    out=mask_hi[:, blk2:blk2 + 128],
                    in_=mask_hi[:, blk2:blk2 + 128],
                    compare_op=mybir.AluOpType.is_ge,
                    fill=0.0, base=-1, pattern=[[1, 128]],
                    channel_multiplier=-1)

            # ---- K projection over halo; zero at padded rows via ind fold -
            kT_sb = actp.tile([128, TH], bf16, tag="kT")

            def k_proj(c0, cw, tag="proj"):
                k_ps = ps.tile([128, 512], f32, tag=tag, name="k_ps",
                               bufs=(4 if tag == "s2" else 2))
                for k in range(8):
                    nc.tensor.matmul(
                        out=k_ps[:, :cw], lhsT=wk_sb[k],
                        rhs=xT_sb[k][:, c0:c0 + cw],
                        start=(k == 0), stop=False)
                nc.tensor.matmul(
                    out=k_ps[:, :cw], lhsT=bkr, rhs=ind_sb[:, c0:c0 + cw],
                    start=False, stop=True)
                nc.vector.tensor_copy(out=kT_sb[:, c0:c0 + cw],
                                      in_=k_ps[:, :cw])

            # ---- V projection (keys on partitions). Layout per u-tile is
            # [V_kv0 (64) | 1 | V_kv1 (64) | 1]: the ones column appended to
            # each kv-slice makes the probs@[V|1] matmul emit the softmax
            # denominator as output column 64 for free. ---------------------
            NU = TH // 128
            v_sb = actp.tile([128, NU * 130], bf16, tag="V")
            v_view = v_sb.rearrange("p (u g c) -> p u g c", u=NU, g=2)
            nc.vector.memset(v_view[:, :, :, 64:65], 1.0)

            def v_proj(ut, tag="proj"):
                v_ps = ps.tile([128, 512], f32, tag=tag, name="v_ps",
                               bufs=(4 if tag == "s2" else 2))
                for k in range(8):
                    nc.tensor.matmul(
                        out=v_ps[:, :KV * D],
                        lhsT=xT_sb[k][:, ut * 128:(ut + 1) * 128],
                        rhs=wv_sb[k], start=(k == 0), stop=False)
                nc.tensor.matmul(
                    out=v_ps[:, :KV * D],
                    lhsT=ind_sb[:, ut * 128:(ut + 1) * 128], rhs=bvr,
                    start=False, stop=True)
                nc.vector.tensor_copy(
                    out=v_view[:, ut, :, 0:64],
                    in_=v_ps[:, :KV * D].rearrange("p (g c) -> p g c", g=2))

            # ---- Q projection: qT tile g holds heads (2g, 2g+1) along the
            # free dim and heads (+8) on the upper partition half ------------
            qT_sb = []
            for g in range(2):
                t_qt = actp.tile([128, 4 * T], bf16, tag=f"qT{g}",
                                 name=f"qT{g}")
                qT_sb.append(t_qt)

            Identity = mybir.ActivationFunctionType.Identity

            def q_proj(m, n, on_act=False, tag="proj"):
                q_ps = ps.tile([128, 512], f32, tag=tag, name="q_ps",
                               bufs=(4 if tag == "s2" else 2))
                for k in range(8):
                    nc.tensor.matmul(
                        out=q_ps,
                        lhsT=wq_sbm[m][:, k * 128:(k + 1) * 128],
                        rhs=xT_sb[k][:, HW + n * 512: HW + (n + 1) * 512],
                        start=(k == 0), stop=(k == 7))
                off = (m % 4) * T + n * 512
                dst = qT_sb[m // 4][:, off:off + 512]
                if on_act:
                    # ScalarE is idle before the attention phase starts
                    nc.scalar.activation(out=dst, in_=q_ps, func=Identity,
                                         bias=bq_sb[:, m:m + 1], scale=1.0)
                else:
                    nc.vector.tensor_scalar_add(out=dst, in0=q_ps,
                                                scalar1=bq_sb[:, m:m + 1])

            # ---- pre-attention work (data-arrival ordered). The attention
            # s2 PSUM banks are idle here: borrow them so the projection
            # pipeline is 4 deep instead of 2 and never waits on copies. ----
            k_proj(0, 512, tag="s2")
            k_proj(512, XSPLIT - 512, tag="s2")
            for ut in range(3):
                v_proj(ut, tag="s2")
            for m in range(8):
                q_proj(m, 0, on_act=True, tag="s2")

            # ---- attention + output transpose + (skewed) out-projection ---
            attnT = actp.tile([128, 8 * T], bf16, tag="attnT")
            attnT_v = attnT.rearrange("p (k t) -> p k t", k=8)

            def out_proj_half(mt, n):
                out_t = attnp.tile([128, 512], bf16, tag="outt")
                o2 = ps.tile([128, 512], f32, tag="proj", name="o2_ps")
                for k in range(8):
                    nc.tensor.matmul(
                        out=o2,
                        lhsT=attnT[:, k * T + mt * 128:
                                   k * T + (mt + 1) * 128],
                        rhs=wo_sb[k][:, n * 512:(n + 1) * 512],
                        start=(k == 0), stop=(k == 7))
                nc.vector.tensor_copy(out=out_t, in_=o2)
                nc.sync.dma_start(
                    out=out[mt * 128:(mt + 1) * 128, n * 512:(n + 1) * 512],
                    in_=out_t)

            def out_proj(mt):
                out_proj_half(mt, 0)
                out_proj_half(mt, 1)

            # PE filler work per (tile, slot): slots 0/1 run between the two
            # attention head-groups (covering the exp latency with
            # independent matmuls), slot 2 after the transposes.
            fillers = {
                0: ([lambda: v_proj(3)], [lambda: v_proj(4)],
                    [lambda: k_proj(XSPLIT, 512)]),
                1: ([lambda: k_proj(XSPLIT + 512, TH - XSPLIT - 512)],
                    [lambda: v_proj(5)], [lambda: out_proj(0)]),
                2: ([lambda: q_proj(0, 1), lambda: q_proj(1, 1)],
                    [lambda: v_proj(6), lambda: q_proj(2, 1)],
                    [lambda: q_proj(3, 1), lambda: out_proj(1)]),
                3: ([lambda: q_proj(4, 1), lambda: q_proj(5, 1)],
                    [lambda: v_proj(7), lambda: q_proj(6, 1)],
                    [lambda: q_proj(7, 1), lambda: out_proj(2)]),
                4: ([lambda: out_proj_half(3, 0), lambda: v_proj(8)],
                    [lambda: out_proj_half(3, 1), lambda: v_proj(9)], []),
                5: ([lambda: out_proj_half(4, 0)],
                    [lambda: out_proj_half(4, 1)], []),
                6: ([lambda: out_proj_half(5, 0)],
                    [lambda: out_proj_half(5, 1)], []),
                7: ([lambda: out_proj_half(6, 0)],
                    [lambda: out_proj_half(6, 1)], []),
            }

            for mt in range(8):
                qcol = mt * 128
                u0 = qcol  # halo col of first attended key
                attn_t = attnp.tile([128, DIM], bf16, tag="attn")
                for gg in range(2):
                    qv = qT_sb[gg].rearrange("p (i t) -> p i t", i=4)
                    p2s = []

                    def score_chunk(j):
                        # per (key-chunk, kv-half) scores in 1-bank PSUM
                        # tiles; the two halves stay row-packed on the PE
                        # (concurrent), but get separate exps so the
                        # exp->PV latency quantum is halved.
                        pj = {}
                        for half in range(2):
                            s2h = ps.tile([128, 512], f32, tag="s2", bufs=4,
                                          name="s2h")
                            nc.tensor.matmul(
                                out=s2h,
                                lhsT=kT_sb[half * 64:(half + 1) * 64,
                                           u0 + j * 128:u0 + (j + 1) * 128],
                                rhs=qv[half * 64:(half + 1) * 64, :,
                                       qcol:qcol + 128],
                                start=True, stop=True,
                                tile_position=(64 * half, 0))
                            p2h = attnp.tile([128, 512], bf16, tag="P",
                                             bufs=12, name="p2h")
                            nc.scalar.activation(out=p2h, in_=s2h, func=Exp)
                            if j == 0:
                                nc.vector.tensor_mul(p2h, p2h, mask_lo)
                            elif j == 2:
                                nc.vector.tensor_mul(p2h, p2h, mask_hi)
                            pj[half] = p2h
                        p2s.append(pj)

                    score_chunk(0)
                    score_chunk(1)
                    score_chunk(2)
                    # independent PE work covers the exp latency before PV
                    for f in fillers[mt][gg]:
                        f()
                    for half in range(2):
                        # 4 heads share one PSUM bank: [a, 0:64]=attn out,
                        # [a, 64]=softmax denominator.
                        o4 = ps.tile([128, 260], f32, tag="o4", bufs=2,
                                     name="o4")
                        o4v = o4.rearrange("p (a c) -> p a c", a=4)
                        for a in range(4):
                            for j in range(3):
                                nc.tensor.matmul(
                                    out=o4v[:, a, :],
                                    lhsT=p2s[j][half][:, a * 128:
                                                      (a + 1) * 128],
                                    rhs=v_view[:, mt + j, half, 0:65],
                                    start=(j == 0), stop=(j == 2))
                        rc4 = attnp.tile([128, 4], f32, tag="rc4", bufs=4,
                                         name="rc4")
                        nc.vector.reciprocal(out=rc4[:, :].unsqueeze(2),
                                             in_=o4v[:, :, 64:65])
                        hbase = (4 * gg + 8 * half) * 64
                        dst = attn_t[:, hbase:hbase + 256].rearrange(
                            "p (a d) -> p a d", a=4)
                        nc.vector.tensor_mul(
                            dst, o4v[:, :, 0:64],
                            rc4[:, :].unsqueeze(2).broadcast_to([128, 4, 64]))
                # transpose attn rows (t) x cols (hd) -> attnT k-tiles
                for g in range(3):
                    kcnt = 3 if g < 2 else 2
                    at_ps = ps.tile([128, 384], bf16, tag="o4", bufs=2,
                                    name="at_ps")
                    for jj in range(kcnt):
                        kk = g * 3 + jj
                        nc.tensor.matmul(
                            out=at_ps[:, jj * 128:(jj + 1) * 128],
                            lhsT=attn_t[:, kk * 128:(kk + 1) * 128],
                            rhs=ident, is_transpose=True,
                            start=(jj == 0), stop=(jj == kcnt - 1))
                    src = at_ps[:, :kcnt * 128].rearrange(
                        "p (j c) -> p j c", j=kcnt)
                    dst = attnT_v[:, g * 3:g * 3 + kcnt, qcol:qcol + 128]
                    nc.vector.tensor_copy(out=dst, in_=src)
                for f in fillers[mt][2]:
                    f()
            out_proj(7)

    nc.compile()
    return nc


def _host_prep(x, Wq, bq, Wk, bk, Wv, bv, Wo, bo):
    import ml_dtypes
    bf16 = ml_dtypes.bfloat16

    # permute Wq/bq columns so qT m-tile holds head m on partitions 0-63 and
    # head m+8 on partitions 64-127 (enables row-packed score matmuls)
    idx = np.empty(DIM, dtype=np.int64)
    for m in range(8):
        for j in range(128):
            h = m if j < 64 else m + 8
            idx[m * 128 + j] = h * D + (j % 64)
    wq_p = Wq[:, idx]
    # m-major layout: row m*128+p, col k*128+c holds Wq_perm[k*128+p, m*128+c]
    wq_p = np.ascontiguousarray(
        wq_p.reshape(8, 128, 8, 128).transpose(2, 1, 0, 3).reshape(
            DIM, DIM)).astype(bf16)
    bq_p = bq[idx].astype(np.float32).reshape(8, 128).T.copy()  # (128, 8)
    wkv_b = np.ascontiguousarray(
        np.concatenate([Wk, Wv], axis=1)).astype(bf16)
    wo_b = np.ascontiguousarray(Wo).astype(bf16)

    ident_h = np.eye(128, dtype=np.float32).astype(bf16)
    r, c = np.arange(128)[:, None], np.arange(128)[None, :]
    mask_h = np.concatenate(
        [np.tile((r >= c).astype(np.float32), (1, 8)),
         np.tile((r < c).astype(np.float32), (1, 8))], axis=1).astype(bf16)

    in_maps = []
    for c in range(NCORES):
        b, qt = c // QT, c % QT
        lo, hi = qt * T - HW, qt * T + T + HW
        xs = np.zeros((TH, DIM), dtype=np.float32)
        s0, s1 = max(lo, 0), min(hi, S)
        xs[s0 - lo:s1 - lo] = x[b, s0:s1]
        crow = np.zeros((1, 2560), dtype=np.float32)
        crow[0, 0:128] = bk
        crow[0, 128:256] = bv
        crow[0, 256:1280] = bo
        crow[0, 1280 + (s0 - lo):1280 + (s1 - lo)] = 1.0
        in_maps.append({
            "xT": np.ascontiguousarray(xs.T).astype(bf16),
            "Wq": wq_p, "Wkv": wkv_b, "Wo": wo_b,
            "bqc": bq_p, "crow": crow.astype(bf16),
            "identd": ident_h, "maskd": mask_h,
        })
    return in_maps


def kernel(x, Wq, bq, Wk, bk, Wv, bv, Wo, bo):
    from concourse.bass_utils import run_bass_kernel_spmd

    x, Wq, bq, Wk, bk, Wv, bv, Wo, bo = (
        np.asarray(a, dtype=np.float32)
        for a in (x, Wq, bq, Wk, bk, Wv, bv, Wo, bo))
    nc = _build_nc()
    in_maps = _host_prep(x, Wq, bq, Wk, bk, Wv, bv, Wo, bo)
    res = run_bass_kernel_spmd(nc, in_maps, core_ids=list(range(NCORES)))
    out = np.empty((B, S, DIM), dtype=np.float32)
    bo32 = bo.astype(np.float32)
    for c in range(NCORES):
        b, qt = c // QT, c % QT
        out[b, qt * T:(qt + 1) * T] = (
            res.results[c]["out"].astype(np.float32) + bo32)
    return out


# revision 50
# speedup vs baseline: 1.0109x; 1.0109x over previous
"""Trainium2 Bass kernel for sliding-window GQA attention block.

Reference computation (B=2, S=4096, DIM=1024, H=16 q-heads, KV=2 kv-heads,
D=64, W=256 window):
    q = x@Wq + bq ; k = x@Wk + bk ; v = x@Wv + bv        (GQA repeat kv x8)
    local attention: query t attends keys [t-128, t+128) (zero-padded edges,
    no 1/sqrt(d) scaling), softmax, out = probs@v
    y = out@Wo + bo   (bo is added on the host after the gather)

Sharding: 8 cores = batch(2) x seq-quarter(4). Each core computes 1024
query rows end-to-end (all 16 heads) from a 1280-row haloed x slice.
No cross-core communication; host pads/transposes/gathers.

On-device pipeline per core (all matmuls bf16, fp32 PSUM accumulation):
  QKV projections (Q bias via per-partition add during the PSUM->SBUF
  copy; K/V biases via K=1 indicator-row matmuls that also zero K,V at
  padded halo rows) -> scores computed directly TRANSPOSED (S^T[u,t],
  keys on partitions; the two kv-halves of each key-chunk row-packed via
  tile_position so they run concurrently on the PE, each into its own
  1-bank PSUM tile) -> per-half exp on ScalarE (its only job; halves the
  exp->PV latency quantum) -> 0/1 band-mask multiply on VectorE for the
  two triangular chunks only -> probs @ [V|1] grouped 4 heads per PSUM
  bank (the ones column appended to V emits the softmax denominator at
  column 64 of each head's 65-col group) -> one strided reciprocal + one
  partition-broadcast tensor_tensor multiply normalizes 4 heads at once
  -> PE-transpose of the 128x128 attn blocks -> out-projection streamed
  per 512-col half with its own output DMA.

Scheduling: input DMAs are big multi-chunk transfers spread over the 3
DMA-capable queues, ordered so K/V-projection data lands first; Wq is
stored m-major on the host and arrives as 8 contiguous per-tile slices
so each qT projection starts as soon as its own slice lands. Identity
comes from the host; band masks build on gpsimd AFTER its DMA issues.
Self-contained junk matmuls warm the PE HAM clock gate during the DMA
wait. The pre-attention projections borrow the (then idle) attention
score PSUM banks for a 4-deep pipeline. Inside the attention loop the
out-projection of tile mt-1 and the remaining JIT projections run as
fillers between the score matmuls and PV of each head-group, covering
the serial exp latency; only out_proj(7) remains after the last tile.
"""

import functools
import numpy as np

B, S, DIM = 2, 4096, 1024
H, KV, D = 16, 2, 64
W, HW = 256, 128
NCORES = 8
QT = 4           # sequence quarters
T = S // QT      # 1024 query rows per core
TH = T + 2 * HW  # 1280 haloed rows
XSPLIT = 640     # xT column slab boundary


@functools.lru_cache(maxsize=1)
def _build_nc():
    import concourse.bacc as bacc
    import concourse.tile as tile
    from concourse import mybir

    f32 = mybir.dt.float32
    bf16 = mybir.dt.bfloat16
    Exp = mybir.ActivationFunctionType.Exp

    nc = bacc.Bacc("TRN2", target_bir_lowering=False, debug=False)

    xT = nc.dram_tensor("xT", [DIM, TH], bf16, kind="ExternalInput")
    wq = nc.dram_tensor("Wq", [DIM, DIM], bf16, kind="ExternalInput")
    wkv = nc.dram_tensor("Wkv", [DIM, 2 * KV * D], bf16, kind="ExternalInput")
    wo = nc.dram_tensor("Wo", [DIM, DIM], bf16, kind="ExternalInput")
    bqc = nc.dram_tensor("bqc", [128, 8], f32, kind="ExternalInput")
    # [bk (128) | bv (128) | bo (1024) | ind (1280)]
    crow = nc.dram_tensor("crow", [1, 2560], bf16, kind="ExternalInput")
    identd = nc.dram_tensor("identd", [128, 128], bf16, kind="ExternalInput")
    out = nc.dram_tensor("out", [T, DIM], bf16, kind="ExternalOutput")

    with tile.TileContext(nc) as tc:
        with tc.tile_pool(name="const", bufs=1) as const, \
             tc.tile_pool(name="w", bufs=1) as wpool, \
             tc.tile_pool(name="act", bufs=1) as actp, \
             tc.tile_pool(name="attn", bufs=2) as attnp, \
             tc.tile_pool(name="ps", bufs=2, space="PSUM") as ps:

            # ---- PE warmup: keeps the HAM clock gate open while the input
            # DMAs stream. Uses a locally-memset zero tile so the warmup has
            # no DMA dependency and starts right after the preamble. --------
            junk_rhs = const.tile([128, 512], bf16, tag="junk_rhs")
            nc.vector.memset(junk_rhs, 0.0)
            junk_ps = ps.tile([128, 512], f32, tag="proj", name="junk")
            for _ in range(28):
                nc.tensor.matmul(out=junk_ps, lhsT=junk_rhs[:, 0:128],
                                 rhs=junk_rhs, start=True, stop=True)

            # ---- DMAs: few big multi-chunk transfers, explicit queues -----
            bq_sb = const.tile([128, 8], f32, tag="bq")
            crow_sb = const.tile([1, 2560], bf16, tag="crow")
            bkr = crow_sb[:, 0:KV * D]
            bvr = crow_sb[:, KV * D:2 * KV * D]
            ind_sb = crow_sb[:, 1280:1280 + TH]

            wkv_big = wpool.tile([128, 8 * 2 * KV * D], bf16, tag="wkv")
            wkv_v = wkv_big.rearrange("p (k c) -> p k c", k=8)
            wk_sb = [wkv_v[:, k, 0:KV * D] for k in range(8)]
            wv_sb = [wkv_v[:, k, KV * D:2 * KV * D] for k in range(8)]
            xT_big = wpool.tile([128, 8 * TH], bf16, tag="xT")
            xT_v = xT_big.rearrange("p (k c) -> p k c", k=8)
            xT_sb = [xT_v[:, k, :] for k in range(8)]
            # per-m Wq tiles: both DMA sides fully contiguous (cheap issue)
            wq_sbm = [wpool.tile([128, 8 * 128], bf16, tag=f"wq{m}",
                                 name=f"wq{m}") for m in range(8)]
            wo_big = wpool.tile([128, 8 * DIM], bf16, tag="wo")
            wo_v = wo_big.rearrange("p (k c) -> p k c", k=8)
            wo_sb = [wo_v[:, k, :] for k in range(8)]

            xT_r = xT.rearrange("(k p) c -> p k c", k=8)
            wo_r = wo.rearrange("(k p) c -> p k c", k=8)
            wkv_r = wkv.rearrange("(k p) c -> p k c", k=8)

            # Wq arrives as per-m column slices so each qT tile's projection
            # can start as soon as its own 256KB lands (overlaps the DMA).
            # The host stores Wq m-major ([m, p, k, c]) so each slice is one
            # fully contiguous 256KB read with 2KB per-partition lines.
            wq_mr = wq.rearrange("(m p) c -> m p c", m=8)

            def wq_m(eng, m):
                eng.dma_start(out=wq_sbm[m], in_=wq_mr[m])

            # sync queue: xT slab halves + wq m-slices + slab1 + wo (k 0-3)
            nc.sync.dma_start(out=xT_v[:, 0:4, 0:XSPLIT],
                              in_=xT_r[:, 0:4, 0:XSPLIT])
            wq_m(nc.sync, 0)
            wq_m(nc.sync, 2)
            wq_m(nc.sync, 4)
            nc.sync.dma_start(out=xT_v[:, 0:4, XSPLIT:TH],
                              in_=xT_r[:, 0:4, XSPLIT:TH])
            nc.sync.dma_start(out=wo_v[:, 0:4, :], in_=wo_r[:, 0:4, :])
            # scalar queue: the other halves
            nc.scalar.dma_start(out=xT_v[:, 4:8, 0:XSPLIT],
                                in_=xT_r[:, 4:8, 0:XSPLIT])
            wq_m(nc.scalar, 1)
            wq_m(nc.scalar, 3)
            wq_m(nc.scalar, 5)
            nc.scalar.dma_start(out=xT_v[:, 4:8, XSPLIT:TH],
                                in_=xT_r[:, 4:8, XSPLIT:TH])
            nc.scalar.dma_start(out=wo_v[:, 4:8, :], in_=wo_r[:, 4:8, :])
            # gpsimd queue: K-projection-critical data first. ident is
            # only needed by the transposes (~30us in), so it goes last.
            ident = const.tile([128, 128], bf16, tag="ident")
            nc.gpsimd.dma_start(out=crow_sb, in_=crow[:, :])
            nc.gpsimd.dma_start(out=bq_sb, in_=bqc[:, :])
            nc.gpsimd.dma_start(out=wkv_v[:, :, :], in_=wkv_r)
            # gpsimd is done by ~13us while sync/scalar still stream: give
            # it the last two Wq slices so the attention start isn't paced
            # by a 5th transfer on the busy queues
            wq_m(nc.gpsimd, 6)
            wq_m(nc.gpsimd, 7)
            nc.gpsimd.dma_start(out=ident, in_=identd[:, :])

            # 0/1 window masks, transposed (key r, query c) orientation, for
            # the two triangular chunks; both kv-halves side by side.
            # j=0 chunk: valid where r >= c; j=2 chunk: valid where r < c.
            # Built on gpsimd AFTER its DMA issues (the engine is idle then).
            masks = const.tile([128, 1024], bf16, tag="masks")
            mask_lo = masks[:, 0:512]
            mask_hi = masks[:, 512:1024]
            nc.gpsimd.memset(mask_lo, 1.0)
            nc.gpsimd.memset(mask_hi, 1.0)
            for blk2 in range(0, 512, 128):
                nc.gpsimd.affine_select(
                    out=mask_lo[:, blk2:blk2 + 128],
                    in_=mask_lo[:, blk2:blk2 + 128],
                    compare_op=mybir.AluOpType.is_ge,
                    fill=0.0, base=0, pattern=[[-1, 128]],
                    channel_multiplier=1)
                nc.gpsimd.affine_select(
                    out=mask_hi[:, blk2:blk2 + 128],
                    in_=mask_hi[:, blk2:blk2 + 128],
                    compare_op=mybir.AluOpType.is_ge,
                    fill=0.0, base=-1, pattern=[[1, 128]],
                    channel_multiplier=-1)

            # ---- K projection over halo; zero at padded rows via ind fold -
            kT_sb = actp.tile([128, TH], bf16, tag="kT")

            def k_proj(c0, cw, tag="proj"):
                k_ps = ps.tile([128, 512], f32, tag=tag, name="k_ps",
                               bufs=(4 if tag == "s2" else 2))
                for k in range(8):
                    nc.tensor.matmul(
                        out=k_ps[:, :cw], lhsT=wk_sb[k],
                        rhs=xT_sb[k][:, c0:c0 + cw],
                        start=(k == 0), stop=False)
                nc.tensor.matmul(
                    out=k_ps[:, :cw], lhsT=bkr, rhs=ind_sb[:, c0:c0 + cw],
                    start=False, stop=True)
                nc.vector.tensor_copy(out=kT_sb[:, c0:c0 + cw],
                                      in_=k_ps[:, :cw])

            # ---- V projection (keys on partitions). Layout per u-tile is
            # [V_kv0 (64) | 1 | V_kv1 (64) | 1]: the ones column appended to
            # each kv-slice makes the probs@[V|1] matmul emit the softmax
            # denominator as output column 64 for free. ---------------------
            NU = TH // 128
            v_sb = actp.tile([128, NU * 130], bf16, tag="V")
            v_view = v_sb.rearrange("p (u g c) -> p u g c", u=NU, g=2)
            nc.vector.memset(v_view[:, :, :, 64:65], 1.0)

            def v_proj(ut, tag="proj"):
                v_ps = ps.tile([128, 512], f32, tag=tag, name="v_ps",
                               bufs=(4 if tag == "s2" else 2))
                for k in range(8):
                    nc.tensor.matmul(
                        out=v_ps[:, :KV * D],
                        lhsT=xT_sb[k][:, ut * 128:(ut + 1) * 128],
                        rhs=wv_sb[k], start=(k == 0), stop=False)
                nc.tensor.matmul(
                    out=v_ps[:, :KV * D],
                    lhsT=ind_sb[:, ut * 128:(ut + 1) * 128], rhs=bvr,
                    start=False, stop=True)
                nc.vector.tensor_copy(
                    out=v_view[:, ut, :, 0:64],
                    in_=v_ps[:, :KV * D].rearrange("p (g c) -> p g c", g=2))

            # ---- Q projection: qT tile g holds heads (2g, 2g+1) along the
            # free dim and heads (+8) on the upper partition half ------------
            qT_sb = []
            for g in range(2):
                t_qt = actp.tile([128, 4 * T], bf16, tag=f"qT{g}",
                                 name=f"qT{g}")
                qT_sb.append(t_qt)

            Identity = mybir.ActivationFunctionType.Identity

            def q_proj(m, n, on_act=False, tag="proj"):
                q_ps = ps.tile([128, 512], f32, tag=tag, name="q_ps",
                               bufs=(4 if tag == "s2" else 2))
                for k in range(8):
                    nc.tensor.matmul(
                        out=q_ps,
                        lhsT=wq_sbm[m][:, k * 128:(k + 1) * 128],
                        rhs=xT_sb[k][:, HW + n * 512: HW + (n + 1) * 512],
                        start=(k == 0), stop=(k == 7))
                off = (m % 4) * T + n * 512
                dst = qT_sb[m // 4][:, off:off + 512]
                if on_act:
                    # ScalarE is idle before the attention phase starts
                    nc.scalar.activation(out=dst, in_=q_ps, func=Identity,
                                         bias=bq_sb[:, m:m + 1], scale=1.0)
                else:
                    nc.vector.tensor_scalar_add(out=dst, in0=q_ps,
                                                scalar1=bq_sb[:, m:m + 1])

            # ---- pre-attention work (data-arrival ordered). The attention
            # s2 PSUM banks are idle here: borrow them so the projection
            # pipeline is 4 deep instead of 2 and never waits on copies. ----
            k_proj(0, 512, tag="s2")
            k_proj(512, XSPLIT - 512, tag="s2")
            for ut in range(3):
                v_proj(ut, tag="s2")
            for m in range(8):
                q_proj(m, 0, on_act=True, tag="s2")

            # ---- attention + output transpose + (skewed) out-projection ---
            attnT = actp.tile([128, 8 * T], bf16, tag="attnT")
            attnT_v = attnT.rearrange("p (k t) -> p k t", k=8)

            def out_proj_half(mt, n):
                out_t = attnp.tile([128, 512], bf16, tag="outt")
                o2 = ps.tile([128, 512], f32, tag="proj", name="o2_ps")
                for k in range(8):
                    nc.tensor.matmul(
                        out=o2,
                        lhsT=attnT[:, k * T + mt * 128:
                                   k * T + (mt + 1) * 128],
                        rhs=wo_sb[k][:, n * 512:(n + 1) * 512],
                        start=(k == 0), stop=(k == 7))
                nc.vector.tensor_copy(out=out_t, in_=o2)
                nc.sync.dma_start(
                    out=out[mt * 128:(mt + 1) * 128, n * 512:(n + 1) * 512],
                    in_=out_t)

            def out_proj(mt):
                out_proj_half(mt, 0)
                out_proj_half(mt, 1)

            # PE filler work per (tile, slot): slots 0/1 run between the two
            # attention head-groups (covering the exp latency with
            # independent matmuls), slot 2 after the transposes.
            fillers = {
                0: ([lambda: v_proj(3)], [lambda: v_proj(4)],
                    [lambda: k_proj(XSPLIT, 512)]),
                1: ([lambda: k_proj(XSPLIT + 512, TH - XSPLIT - 512)],
                    [lambda: v_proj(5)], [lambda: out_proj(0)]),
                2: ([lambda: q_proj(0, 1), lambda: q_proj(1, 1)],
                    [lambda: v_proj(6), lambda: q_proj(2, 1)],
                    [lambda: q_proj(3, 1), lambda: out_proj(1)]),
                3: ([lambda: q_proj(4, 1), lambda: q_proj(5, 1)],
                    [lambda: v_proj(7), lambda: q_proj(6, 1)],
                    [lambda: q_proj(7, 1), lambda: out_proj(2)]),
                4: ([lambda: out_proj_half(3, 0), lambda: v_proj(8)],
                    [lambda: out_proj_half(3, 1), lambda: v_proj(9)], []),
                5: ([lambda: out_proj_half(4, 0)],
                    [lambda: out_proj_half(4, 1)], []),
                6: ([lambda: out_proj_half(5, 0)],
                    [lambda: out_proj_half(5, 1)], []),
                7: ([lambda: out_proj_half(6, 0)],
                    [lambda: out_proj_half(6, 1)], []),
            }

            for mt in range(8):
                qcol = mt * 128
                u0 = qcol  # halo col of first attended key
                attn_t = attnp.tile([128, DIM], bf16, tag="attn")
                for gg in range(2):
                    qv = qT_sb[gg].rearrange("p (i t) -> p i t", i=4)
                    p2s = []

                    def score_chunk(j):
                        # per (key-chunk, kv-half) scores in 1-bank PSUM
                        # tiles; the two halves stay row-packed on the PE
                        # (concurrent), but get separate exps so the
                        # exp->PV latency quantum is halved.
                        pj = {}
                        for half in range(2):
                            s2h = ps.tile([128, 512], f32, tag="s2", bufs=4,
                                          name="s2h")
                            nc.tensor.matmul(
                                out=s2h,
                                lhsT=kT_sb[half * 64:(half + 1) * 64,
                                           u0 + j * 128:u0 + (j + 1) * 128],
                                rhs=qv[half * 64:(half + 1) * 64, :,
                                       qcol:qcol + 128],
                                start=True, stop=True,
                                tile_position=(64 * half, 0))
                            p2h = attnp.tile([128, 512], bf16, tag="P",
                                             bufs=12, name="p2h")
                            nc.scalar.activation(out=p2h, in_=s2h, func=Exp)
                            if j == 0:
                                nc.vector.tensor_mul(p2h, p2h, mask_lo)
                            elif j == 2:
                                nc.vector.tensor_mul(p2h, p2h, mask_hi)
                            pj[half] = p2h
                        p2s.append(pj)

                    score_chunk(0)
                    score_chunk(1)
                    score_chunk(2)
                    # independent PE work covers the exp latency before PV
                    for f in fillers[mt][gg]:
                        f()
                    for half in range(2):
                        # 4 heads share one PSUM bank: [a, 0:64]=attn out,
                        # [a, 64]=softmax denominator.
                        o4 = ps.tile([128, 260], f32, tag="o4", bufs=2,
                                     name="o4")
                        o4v = o4.rearrange("p (a c) -> p a c", a=4)
                        for a in range(4):
                            for j in range(3):
                                nc.tensor.matmul(
                                    out=o4v[:, a, :],
                                    lhsT=p2s[j][half][:, a * 128:
                                                      (a + 1) * 128],
                                    rhs=v_view[:, mt + j, half, 0:65],
                                    start=(j == 0), stop=(j == 2))
                        rc4 = attnp.tile([128, 4], f32, tag="rc4", bufs=4,
                                         name="rc4")
                        nc.vector.reciprocal(out=rc4[:, :].unsqueeze(2),
                                             in_=o4v[:, :, 64:65])
                        hbase = (4 * gg + 8 * half) * 64
                        dst = attn_t[:, hbase:hbase + 256].rearrange(
                            "p (a d) -> p a d", a=4)
                        nc.vector.tensor_mul(
                            dst, o4v[:, :, 0:64],
                            rc4[:, :].unsqueeze(2).broadcast_to([128, 4, 64]))
                # transpose attn rows (t) x cols (hd) -> attnT k-tiles
                for g in range(3):
                    kcnt = 3 if g < 2 else 2
                    at_ps = ps.tile([128, 384], bf16, tag="o4", bufs=2,
                                    name="at_ps")
                    for jj in range(kcnt):
                        kk = g * 3 + jj
                        nc.tensor.matmul(
                            out=at_ps[:, jj * 128:(jj + 1) * 128],
                            lhsT=attn_t[:, kk * 128:(kk + 1) * 128],
                            rhs=ident, is_transpose=True,
                            start=(jj == 0), stop=(jj == kcnt - 1))
                    src = at_ps[:, :kcnt * 128].rearrange(
                        "p (j c) -> p j c", j=kcnt)
                    dst = attnT_v[:, g * 3:g * 3 + kcnt, qcol:qcol + 128]
                    nc.vector.tensor_copy(out=dst, in_=src)
                for f in fillers[mt][2]:
                    f()
            out_proj(7)

    nc.compile()
    return nc


def _host_prep(x, Wq, bq, Wk, bk, Wv, bv, Wo, bo):
    import ml_dtypes
    bf16 = ml_dtypes.bfloat16

    # permute Wq/bq columns so qT m-tile holds head m on partitions 0-63 and
    # head m+8 on partitions 64-127 (enables row-packed score matmuls)
    idx = np.empty(DIM, dtype=np.int64)
    for m in range(8):
        for j in range(128):
            h = m if j < 64 else m + 8
            idx[m * 128 + j] = h * D + (j % 64)
    wq_p = Wq[:, idx]
    # m-major layout: row m*128+p, col k*128+c holds Wq_perm[k*128+p, m*128+c]
    wq_p = np.ascontiguousarray(
        wq_p.reshape(8, 128, 8, 128).transpose(2, 1, 0, 3).reshape(
            DIM, DIM)).astype(bf16)
    bq_p = bq[idx].astype(np.float32).reshape(8, 128).T.copy()  # (128, 8)
    wkv_b = np.ascontiguousarray(
        np.concatenate([Wk, Wv], axis=1)).astype(bf16)
    wo_b = np.ascontiguousarray(Wo).astype(bf16)

    ident_h = np.eye(128, dtype=np.float32).astype(bf16)
    r, c = np.arange(128)[:, None], np.arange(128)[None, :]
    mask_h = np.concatenate(
        [np.tile((r >= c).astype(np.float32), (1, 8)),
         np.tile((r < c).astype(np.float32), (1, 8))], axis=1).astype(bf16)

    in_maps = []
    for c in range(NCORES):
        b, qt = c // QT, c % QT
        lo, hi = qt * T - HW, qt * T + T + HW
        xs = np.zeros((TH, DIM), dtype=np.float32)
        s0, s1 = max(lo, 0), min(hi, S)
        xs[s0 - lo:s1 - lo] = x[b, s0:s1]
        crow = np.zeros((1, 2560), dtype=np.float32)
        crow[0, 0:128] = bk
        crow[0, 128:256] = bv
        crow[0, 256:1280] = bo
        crow[0, 1280 + (s0 - lo):1280 + (s1 - lo)] = 1.0
        in_maps.append({
            "xT": np.ascontiguousarray(xs.T).astype(bf16),
            "Wq": wq_p, "Wkv": wkv_b, "Wo": wo_b,
            "bqc": bq_p, "crow": crow.astype(bf16),
            "identd": ident_h, "maskd": mask_h,
        })
    return in_maps


def kernel(x, Wq, bq, Wk, bk, Wv, bv, Wo, bo):
    from concourse.bass_utils import run_bass_kernel_spmd

    x, Wq, bq, Wk, bk, Wv, bv, Wo, bo = (
        np.asarray(a, dtype=np.float32)
        for a in (x, Wq, bq, Wk, bk, Wv, bv, Wo, bo))
    nc = _build_nc()
    in_maps = _host_prep(x, Wq, bq, Wk, bk, Wv, bv, Wo, bo)
    res = run_bass_kernel_spmd(nc, in_maps, core_ids=list(range(NCORES)))
    out = np.empty((B, S, DIM), dtype=np.float32)
    bo32 = bo.astype(np.float32)
    for c in range(NCORES):
        b, qt = c // QT, c % QT
        out[b, qt * T:(qt + 1) * T] = (
            res.results[c]["out"].astype(np.float32) + bo32)
    return out
